# revision 5
# baseline (speedup 1.0000x reference)
"""Angular tensor-product basis expansion on 8 Trainium2 NeuronCores.

Input dr [200000, 3] f32 -> output [200000, 1093] f32 where the columns are
the levels of the recursive tensor-product basis: level l has 3^l entries,
entry (j*3+k) of level l = level_{l-1}[j] * dr[k].

The tensor-product basis is symmetric: the level-l entry with base-3 digits
(d1..dl) equals x^a y^b z^c where a,b,c count the digits equal to 0,1,2.
Level l therefore has only C(l+2,2) distinct values; across levels 0..6 the
1093 columns take just 84 distinct monomial values per row, and 4 of those
(1, x, y, z) are the input itself. The device computes exactly the 80
level-2..6 monomials per row (bf16) and the host expands them to the full
1093 fp32 columns with a precomputed index gather during the unshard step --
cutting HBM store traffic per core from 109.7 MB (fp32 full) to 4.0 MB, a
27x reduction on the memory-bound store stream.

Monomial ordering (so each level needs only 3 contiguous strided ops):
  L_1 = [x, y, z];  L_l = [x * L_{l-1} (all)] ++ [y * (last l of L_{l-1})]
                          ++ [z * (last 1 of L_{l-1})]
By induction the a=0 monomials are exactly the trailing l+1 entries of L_l,
so the y-source (a=0 entries of L_{l-1}) is a contiguous tail slice.
Level 2 reads x,y,z straight from the input tile.

Engine split, from measured cost structure (op spacing ~ max(0.95ns *
elems-per-partition, ~200ns issue floor)): the vector engine runs only the
five big x-multiplies per chunk (bandwidth-bound), while gpsimd runs the ten
tiny tail ops (y-tails + z-power chain; issue-floor-bound, negligible
bandwidth, so the engines do not contend). Dependencies run strictly
gpsimd -> vector (the x-op of level l reads level l-1's tail), and gpsimd's
op stream is shorter, so it stays ahead. Chunks are processed in interleaved
pairs on both engines: every RAW wait (ops are not interlocked; each op's
completion tick is what dependents must wait on) lands several ops after its
producer and is already satisfied.

Data-parallel row sharding across 8 cores (25000 rows each, padded to
25088 = 128 partitions * 196 rows). Partition p owns the contiguous row
chunk [p*196, (p+1)*196); store DMAs of finished chunks overlap compute.

Raw Bass (no Tile) so DMA instructions carry at most one semaphore wait --
walrus rejects HWDGE direct DMAs with more than one sync-wait command.
"""

import numpy as np

L_MAX = 6
N_CORES = 8
G = 196  # rows owned by one partition
ROWS_PER_CORE = 128 * G  # 25088
S = [1, 3, 6, 10, 15, 21, 28]  # unique monomials per level
OFF = [0, 0, 0, 6, 16, 31, 52]  # device column offset of level l (l>=2)
U = 80  # stored monomials (levels 2..6)
SIZES = (49, 49, 49, 49)  # rows per chunk; consecutive pairs interleave
VOPS = 5  # vector ops per chunk (x-multiplies, levels 2..6)
GOPS = 10  # gpsimd ops per chunk (y-tails + z-chain, levels 2..6)


def _index_map():
    """Map each of the 1093 reference columns to unique-monomial index 0..83
    (0..3 = [1, x, y, z] host-side; 4+i = device column i)."""
    mono = [[(0, 0, 0)]]
    for l in range(1, L_MAX + 1):
        prev = mono[-1]
        cur = [(a + 1, b, c) for (a, b, c) in prev]
        cur += [(a, b + 1, c) for (a, b, c) in prev[-l:]]
        a, b, c = prev[-1]
        cur += [(a, b, c + 1)]
        mono.append(cur)
    lookup = {t: i for i, t in enumerate(t for lst in mono for t in lst)}
    idx = []
    for l in range(L_MAX + 1):
        for j in range(3**l):
            a = b = c = 0
            for _ in range(l):
                d = j % 3
                j //= 3
                a += d == 0
                b += d == 1
                c += d == 2
            idx.append(lookup[(a, b, c)])
    return np.asarray(idx, dtype=np.intp)


IDX = _index_map()  # [1093] into [1, x, y, z, device cols 0..79]


def _build_nc(sizes=SIZES):
    import concourse.bass as bass
    import concourse.mybir as mybir

    bf16 = mybir.dt.bfloat16
    g = sum(sizes)
    assert g == G
    rows = 128 * g
    starts = np.concatenate([[0], np.cumsum(sizes)[:-1]])
    n_ch = len(sizes)
    assert n_ch % 2 == 0

    nc = bass.Bass()
    dr4 = nc.declare_dram_parameter("dr4", [rows, 4], bf16, isOutput=False)
    out = nc.declare_dram_parameter("out", [rows, U], bf16, isOutput=True)

    # partition-major views: partition p owns rows [p*g, (p+1)*g)
    dr4_v = dr4[:, :].rearrange("(p g) c -> p (g c)", p=128)  # [128, g*4]
    out_v = out[:, :].rearrange("(p g) c -> p (g c)", p=128)  # [128, g*U]

    from contextlib import ExitStack

    with ExitStack() as stack:
        drt = stack.enter_context(nc.sbuf_tensor("drt", [128, g * 4], bf16))
        uq = stack.enter_context(nc.sbuf_tensor("uq", [128, g * U], bf16))
        sem_in = stack.enter_context(nc.semaphore("sem_in"))
        sem_in2 = stack.enter_context(nc.semaphore("sem_in2"))
        sem_out = stack.enter_context(nc.semaphore("sem_out"))
        sem_v = stack.enter_context(nc.semaphore("sem_v"))
        sem_g = stack.enter_context(nc.semaphore("sem_g"))
        block = stack.enter_context(nc.Block(no_gpsimd_drain=True))

        def views(k):
            st, sz = starts[k], sizes[k]
            v = uq[:, st * U : (st + sz) * U].rearrange(
                "p (t c) -> p t c", t=sz
            )
            d = drt[:, st * 4 : (st + sz) * 4].rearrange(
                "p (t c) -> p t c", t=sz
            )
            return v, d, sz

        # per-chunk op completion counts within a pair's op block
        # vector pair order: [A2 a, A2 b, A3 a, A3 b, ..., A6 a, A6 b]
        # gpsimd pair order: [z2 a, z2 b, B2 a, B2 b, z3 a, ..., B6 a, B6 b]
        def vthr(pair, l, who):  # vector ops of chunk through level l
            return 2 * VOPS * pair + 2 * (l - 2) + 1 + who

        def gthr(pair, l, who):  # gpsimd ops of chunk through level l
            return 2 * GOPS * pair + 4 * (l - 2) + 2 + 1 + who

        @block.sync
        def _(sync):
            c0 = (sizes[0] + sizes[1]) * 4  # first-pair input columns
            sync.dma_start(out=drt[:, :c0], in_=dr4_v[:, :c0]).then_inc(
                sem_in, 16
            )
            sync.dma_start(out=drt[:, c0:], in_=dr4_v[:, c0:]).then_inc(
                sem_in2, 16
            )
            for k in range(n_ch):
                st, sz = starts[k], sizes[k]
                sync.wait_ge(sem_v, vthr(k // 2, L_MAX, k % 2))
                sync.wait_ge(sem_g, gthr(k // 2, L_MAX, k % 2))
                src = uq[:, st * U : (st + sz) * U]
                dst = out_v[:, st * U : (st + sz) * U]
                # Completion increments arrive 16x (one per SDMA engine);
                # the final wait below is on the summed total.
                sync.dma_start(out=dst, in_=src).then_inc(sem_out, 16)
            sync.wait_ge(sem_out, 16 * n_ch)

        @block.gpsimd
        def _(gpsimd):
            gpsimd.wait_ge(sem_in, 16)
            for pair in range(n_ch // 2):
                if pair == 1:
                    gpsimd.wait_ge(sem_in2, 16)
                ks = (2 * pair, 2 * pair + 1)
                for l in range(2, L_MAX + 1):
                    for who, k in enumerate(ks):
                        v, d, sz = views(k)
                        # z^l at the last slot of level l's block
                        zo = OFF[l] + S[l] - 1
                        if l == 2:
                            zin = d[:, :, 3:4]
                        else:
                            gpsimd.wait_ge(sem_g, gthr(pair, l - 1, who) - 2)
                            zin = v[:, :, OFF[l - 1] + S[l - 1] - 1 :][
                                :, :, 0:1
                            ]
                        gpsimd.tensor_mul(
                            out=v[:, :, zo : zo + 1], in0=zin, in1=d[:, :, 3:4]
                        ).then_inc(sem_g, 1)
                    for who, k in enumerate(ks):
                        v, d, sz = views(k)
                        # y * (a=0 tail of L_{l-1}: its last l entries);
                        # for l==2 the tail is [y, z] from the input tile
                        o = OFF[l] + S[l - 1]
                        if l == 2:
                            src = d[:, :, 2:4]
                        else:
                            gpsimd.wait_ge(sem_g, gthr(pair, l - 1, who))
                            po = OFF[l - 1] + S[l - 1] - l
                            src = v[:, :, po : po + l]
                        gpsimd.tensor_mul(
                            out=v[:, :, o : o + l],
                            in0=src,
                            in1=d[:, :, 2:3].broadcast_to([128, sz, l]),
                        ).then_inc(sem_g, 1)

        @block.vector
        def _(vector):
            vector.wait_ge(sem_in, 16)
            for pair in range(n_ch // 2):
                if pair == 1:
                    vector.wait_ge(sem_in2, 16)
                ks = (2 * pair, 2 * pair + 1)
                for l in range(2, L_MAX + 1):
                    for who, k in enumerate(ks):
                        v, d, sz = views(k)
                        o, ps = OFF[l], S[l - 1]
                        if l == 2:
                            prev = d[:, :, 1:4]
                        else:
                            # x-op reads all of L_{l-1}: own A-block plus
                            # gpsimd's tail ops of level l-1
                            vector.wait_ge(sem_v, vthr(pair, l - 1, who))
                            vector.wait_ge(sem_g, gthr(pair, l - 1, who))
                            prev = v[:, :, OFF[l - 1] : OFF[l - 1] + ps]
                        vector.tensor_mul(
                            out=v[:, :, o : o + ps],
                            in0=prev,
                            in1=d[:, :, 1:2].broadcast_to([128, sz, ps]),
                        ).then_inc(sem_v, 1)

    return nc


def kernel(dr, _trace=False, _trace_cores=None):
    import ml_dtypes
    from concourse.bass_utils import run_bass_kernel_spmd

    dr = np.asarray(dr, dtype=np.float32)
    n = dr.shape[0]
    # Overlapping shards: core i processes rows [i*step, i*step + 25088) so
    # the 704 rows of pad-to-25088 waste is spread evenly (88 rows per core)
    # instead of all landing on the last core.
    step = n // N_CORES
    assert step <= ROWS_PER_CORE and (N_CORES - 1) * step + ROWS_PER_CORE >= n
    total = (N_CORES - 1) * step + ROWS_PER_CORE
    drb = dr.astype(ml_dtypes.bfloat16)
    dr4 = np.zeros((total, 4), dtype=ml_dtypes.bfloat16)
    dr4[:, 0] = 1.0
    dr4[:n, 1:] = drb

    in_maps = [
        {"dr4": np.ascontiguousarray(dr4[i * step : i * step + ROWS_PER_CORE])}
        for i in range(N_CORES)
    ]
    nc = _build_nc()
    res = run_bass_kernel_spmd(
        nc,
        in_maps,
        core_ids=list(range(N_CORES)),
        trace=_trace,
        trace_cores=_trace_cores,
    )
    kernel.last_result = res
    dev = np.concatenate(
        [res.results[i]["out"][:step] for i in range(N_CORES - 1)]
        + [res.results[N_CORES - 1]["out"][: ROWS_PER_CORE - 88]],
        axis=0,
    )
    # unshard: assemble the 84 unique monomials (host-known [1,x,y,z] +
    # 80 device columns), upcast, and expand to the 1093 output columns
    uniq = np.empty((n, 84), dtype=np.float32)
    uniq[:, 0] = 1.0
    uniq[:, 1:4] = drb.astype(np.float32)  # match device bf16 rounding
    uniq[:, 4:] = np.asarray(dev[:n]).astype(np.float32)
    return uniq[:, IDX]


# revision 6
# speedup vs baseline: 1.1496x; 1.1496x over previous
"""Angular tensor-product basis expansion on 8 Trainium2 NeuronCores.

Input dr [200000, 3] f32 -> output [200000, 1093] f32 where the columns are
the levels of the recursive tensor-product basis: level l has 3^l entries,
entry (j*3+k) of level l = level_{l-1}[j] * dr[k].

The tensor-product basis is symmetric: the level-l entry with base-3 digits
(d1..dl) equals x^a y^b z^c where a,b,c count the digits equal to 0,1,2.
Level l therefore has only C(l+2,2) distinct values; across levels 0..6 the
1093 columns take just 84 distinct monomial values per row, and 4 of those
(1, x, y, z) are the input itself. The device computes exactly the 80
level-2..6 monomials per row (bf16) and the host expands them to the full
1093 fp32 columns with a precomputed index gather during the unshard step --
cutting HBM store traffic per core from 109.7 MB (fp32 full) to 4.0 MB, a
27x reduction on the memory-bound store stream.

Monomial ordering (so each level needs only 3 contiguous strided ops):
  L_1 = [x, y, z];  L_l = [x * L_{l-1} (all)] ++ [y * (last l of L_{l-1})]
                          ++ [z * (last 1 of L_{l-1})]
By induction the a=0 monomials are exactly the trailing l+1 entries of L_l,
so the y-source (a=0 entries of L_{l-1}) is a contiguous tail slice.
Level 2 reads x,y,z straight from the input tile.

Schedule, from the measured DVE cost structure (op spacing ~ max(0.95ns *
elems-per-partition, ~200ns issue floor); a second compute engine only
contends -- vector alone is fastest): rows are processed in PAIRS of
adjacent chunks. Per level the pair runs 4 ops: the tiny z-power and y-tail
ops span the whole pair (halving their count; they sit on the issue floor),
and the big x-multiplies run per chunk so each chunk's store can launch as
soon as its own columns are done. All RAW waits (ops are not interlocked;
each op's completion tick is what dependents wait on) land 4+ ops after
their producer and are therefore pre-satisfied. Store DMAs alternate
between the sync and scalar queues so the final two stores drain in
parallel.

Data-parallel row sharding across 8 cores (25000 rows each, padded to
25088 = 128 partitions * 196 rows). Partition p owns the contiguous row
chunk [p*196, (p+1)*196).

Raw Bass (no Tile) so DMA instructions carry at most one semaphore wait --
walrus rejects HWDGE direct DMAs with more than one sync-wait command.
"""

import numpy as np

L_MAX = 6
N_CORES = 8
G = 196  # rows owned by one partition
ROWS_PER_CORE = 128 * G  # 25088
S = [1, 3, 6, 10, 15, 21, 28]  # unique monomials per level
OFF = [0, 0, 0, 6, 16, 31, 52]  # device column offset of level l (l>=2)
U = 80  # stored monomials (levels 2..6)
SIZES = (70, 70, 28, 28)  # rows per chunk; consecutive pairs share tail ops
POPS = 20  # ops per pair: 5 levels * (z + B + A_a + A_b)


def _index_map():
    """Map each of the 1093 reference columns to unique-monomial index 0..83
    (0..3 = [1, x, y, z] host-side; 4+i = device column i)."""
    mono = [[(0, 0, 0)]]
    for l in range(1, L_MAX + 1):
        prev = mono[-1]
        cur = [(a + 1, b, c) for (a, b, c) in prev]
        cur += [(a, b + 1, c) for (a, b, c) in prev[-l:]]
        a, b, c = prev[-1]
        cur += [(a, b, c + 1)]
        mono.append(cur)
    lookup = {t: i for i, t in enumerate(t for lst in mono for t in lst)}
    idx = []
    for l in range(L_MAX + 1):
        for j in range(3**l):
            a = b = c = 0
            for _ in range(l):
                d = j % 3
                j //= 3
                a += d == 0
                b += d == 1
                c += d == 2
            idx.append(lookup[(a, b, c)])
    return np.asarray(idx, dtype=np.intp)


IDX = _index_map()  # [1093] into [1, x, y, z, device cols 0..79]


def _build_nc(sizes=SIZES):
    import concourse.bass as bass
    import concourse.mybir as mybir

    bf16 = mybir.dt.bfloat16
    g = sum(sizes)
    assert g == G
    rows = 128 * g
    starts = np.concatenate([[0], np.cumsum(sizes)[:-1]])
    n_ch = len(sizes)
    assert n_ch % 2 == 0

    nc = bass.Bass()
    dr4 = nc.declare_dram_parameter("dr4", [rows, 4], bf16, isOutput=False)
    out = nc.declare_dram_parameter("out", [rows, U], bf16, isOutput=True)

    # partition-major views: partition p owns rows [p*g, (p+1)*g)
    dr4_v = dr4[:, :].rearrange("(p g) c -> p (g c)", p=128)  # [128, g*4]
    out_v = out[:, :].rearrange("(p g) c -> p (g c)", p=128)  # [128, g*U]

    from contextlib import ExitStack

    with ExitStack() as stack:
        drt = stack.enter_context(nc.sbuf_tensor("drt", [128, g * 4], bf16))
        uq = stack.enter_context(nc.sbuf_tensor("uq", [128, g * U], bf16))
        sem_in = stack.enter_context(nc.semaphore("sem_in"))
        sem_in2 = stack.enter_context(nc.semaphore("sem_in2"))
        sem_out = stack.enter_context(nc.semaphore("sem_out"))
        sem_out2 = stack.enter_context(nc.semaphore("sem_out2"))
        sem_v = stack.enter_context(nc.semaphore("sem_v"))
        block = stack.enter_context(nc.Block(no_gpsimd_drain=True))

        def views(st, sz):
            v = uq[:, st * U : (st + sz) * U].rearrange(
                "p (t c) -> p t c", t=sz
            )
            d = drt[:, st * 4 : (st + sz) * 4].rearrange(
                "p (t c) -> p t c", t=sz
            )
            return v, d

        # chunk completion op-count: chunk a of pair p completes at its A6
        # (pair index 19), chunk b at 20 (see emission order below)
        def cthr(k):
            return POPS * (k // 2) + 19 + (k % 2)

        def store(q, k, sem):
            st, sz = starts[k], sizes[k]
            q.wait_ge(sem_v, cthr(k))
            q.dma_start(
                out=out_v[:, st * U : (st + sz) * U],
                in_=uq[:, st * U : (st + sz) * U],
            ).then_inc(sem, 16)

        @block.sync
        def _(sync):
            c0 = (sizes[0] + sizes[1]) * 4  # first-pair input columns
            sync.dma_start(out=drt[:, :c0], in_=dr4_v[:, :c0]).then_inc(
                sem_in, 16
            )
            sync.dma_start(out=drt[:, c0:], in_=dr4_v[:, c0:]).then_inc(
                sem_in2, 16
            )
            for k in range(0, n_ch, 2):  # even chunks on the sync queue
                store(sync, k, sem_out)
            sync.wait_ge(sem_out, 16 * (n_ch // 2))

        @block.scalar
        def _(scalar):
            for k in range(1, n_ch, 2):  # odd chunks on the scalar queue
                store(scalar, k, sem_out2)
            scalar.wait_ge(sem_out2, 16 * (n_ch - n_ch // 2))

        @block.vector
        def _(vector):
            vector.wait_ge(sem_in, 16)
            for pair in range(n_ch // 2):
                if pair == 1:
                    vector.wait_ge(sem_in2, 16)
                ka, kb = 2 * pair, 2 * pair + 1
                sta, sza = starts[ka], sizes[ka]
                szp = sza + sizes[kb]  # pair-wide row count (adjacent chunks)
                vp, dp = views(sta, szp)
                va, da = views(sta, sza)
                vb, db = views(starts[kb], sizes[kb])
                base = POPS * pair

                for l in range(2, L_MAX + 1):
                    lb = base + 4 * (l - 3)  # previous level's block base
                    o, ps = OFF[l], S[l - 1]
                    zo = o + ps + l  # z^l slot (last of level l)
                    po = OFF[l - 1]
                    # z^l = z * z^(l-1), pair-wide
                    if l == 2:
                        zin = dp[:, :, 3:4]
                    else:
                        vector.wait_ge(sem_v, lb + 1)
                        pzo = po + S[l - 1] - 1
                        zin = vp[:, :, pzo : pzo + 1]
                    vector.tensor_mul(
                        out=vp[:, :, zo : zo + 1], in0=zin, in1=dp[:, :, 3:4]
                    ).then_inc(sem_v, 1)
                    # y-tail: y * (last l of L_{l-1}), pair-wide
                    if l == 2:
                        ysrc = dp[:, :, 2:4]
                    else:
                        vector.wait_ge(sem_v, lb + 2)
                        pto = po + S[l - 1] - l
                        ysrc = vp[:, :, pto : pto + l]
                    vector.tensor_mul(
                        out=vp[:, :, o + ps : o + ps + l],
                        in0=ysrc,
                        in1=dp[:, :, 2:3].broadcast_to([128, szp, l]),
                    ).then_inc(sem_v, 1)
                    # x * (all of L_{l-1}), per chunk
                    for j, (vv, dd, sz) in enumerate(
                        ((va, da, sza), (vb, db, sizes[kb]))
                    ):
                        if l == 2:
                            prev = dd[:, :, 1:4]
                        else:
                            vector.wait_ge(sem_v, lb + 3 + j)
                            prev = vv[:, :, po : po + ps]
                        vector.tensor_mul(
                            out=vv[:, :, o : o + ps],
                            in0=prev,
                            in1=dd[:, :, 1:2].broadcast_to([128, sz, ps]),
                        ).then_inc(sem_v, 1)

    return nc


def kernel(dr, _trace=False, _trace_cores=None):
    import ml_dtypes
    from concourse.bass_utils import run_bass_kernel_spmd

    dr = np.asarray(dr, dtype=np.float32)
    n = dr.shape[0]
    # Overlapping shards: core i processes rows [i*step, i*step + 25088) so
    # the 704 rows of pad-to-25088 waste is spread evenly (88 rows per core)
    # instead of all landing on the last core.
    step = n // N_CORES
    assert step <= ROWS_PER_CORE and (N_CORES - 1) * step + ROWS_PER_CORE >= n
    total = (N_CORES - 1) * step + ROWS_PER_CORE
    drb = dr.astype(ml_dtypes.bfloat16)
    dr4 = np.zeros((total, 4), dtype=ml_dtypes.bfloat16)
    dr4[:, 0] = 1.0
    dr4[:n, 1:] = drb

    in_maps = [
        {"dr4": np.ascontiguousarray(dr4[i * step : i * step + ROWS_PER_CORE])}
        for i in range(N_CORES)
    ]
    nc = _build_nc()
    res = run_bass_kernel_spmd(
        nc,
        in_maps,
        core_ids=list(range(N_CORES)),
        trace=_trace,
        trace_cores=_trace_cores,
    )
    kernel.last_result = res
    dev = np.concatenate(
        [res.results[i]["out"][:step] for i in range(N_CORES - 1)]
        + [res.results[N_CORES - 1]["out"][: ROWS_PER_CORE - 88]],
        axis=0,
    )
    # unshard: assemble the 84 unique monomials (host-known [1,x,y,z] +
    # 80 device columns), upcast, and expand to the 1093 output columns
    uniq = np.empty((n, 84), dtype=np.float32)
    uniq[:, 0] = 1.0
    uniq[:, 1:4] = drb.astype(np.float32)  # match device bf16 rounding
    uniq[:, 4:] = np.asarray(dev[:n]).astype(np.float32)
    return uniq[:, IDX]


# revision 8
# speedup vs baseline: 1.1890x; 1.0343x over previous
"""Angular tensor-product basis expansion on 8 Trainium2 NeuronCores.

Input dr [200000, 3] f32 -> output [200000, 1093] f32 where the columns are
the levels of the recursive tensor-product basis: level l has 3^l entries,
entry (j*3+k) of level l = level_{l-1}[j] * dr[k].

The tensor-product basis is symmetric: the level-l entry with base-3 digits
(d1..dl) equals x^a y^b z^c where a,b,c count the digits equal to 0,1,2.
Level l therefore has only C(l+2,2) distinct values; across levels 0..6 the
1093 columns take just 84 distinct monomial values per row, and 4 of those
(1, x, y, z) are the input itself. The device computes exactly the 80
level-2..6 monomials per row (bf16) and the host expands them to the full
1093 fp32 columns with a precomputed index gather during the unshard step --
cutting HBM store traffic per core from 109.7 MB (fp32 full) to 4.0 MB, a
27x reduction on the memory-bound store stream.

Monomial ordering (so each level needs only 3 strided DVE ops):
  L_1 = [x, y, z];  L_l = [x * L_{l-1} (all)] ++ [y * (last l of L_{l-1})]
                          ++ [z * (last 1 of L_{l-1})]
By induction the a=0 monomials are exactly the trailing l+1 entries of L_l,
so the y-source (a=0 entries of L_{l-1}) is a contiguous tail slice.
Level 2 reads x,y,z straight from the input tile.

Measured DVE cost: op duration ~ n_runs * (run_len * ~1.05ns + ~2.6ns) with
a ~170ns issue floor, where a "run" is the op's innermost contiguous span.
So each chunk's scratch tile is MONOMIAL-MAJOR ([partition, monomial, row]):
every op's inner run is the row dimension (28..70 elems), not the 1..21
monomials a row-major layout would give -- this matters most for the tiny
z-power and y-tail ops, which in row-major cost ~2.6-5ns per element.
Chunks are processed in interleaved pairs so every RAW wait (ops are not
interlocked; each op's completion tick is what dependents wait on) lands
4+ ops after its producer and is pre-satisfied. Store DMAs are contiguous
per-partition dumps of the tile (the host untangles the per-chunk transpose
during the gather), alternating between the sync and scalar DMA queues,
which together sustain >400 GB/s. A second compute engine does not help:
vector and gpsimd contend for the same bandwidth and gpsimd's per-op cost
is ~3x.

Data-parallel row sharding across 8 cores (25000 rows each, padded to
25088 = 128 partitions * 196 rows). Partition p owns the contiguous row
chunk [p*196, (p+1)*196).

Raw Bass (no Tile) so DMA instructions carry at most one semaphore wait --
walrus rejects HWDGE direct DMAs with more than one sync-wait command.
"""

import numpy as np

L_MAX = 6
N_CORES = 8
G = 196  # rows owned by one partition
ROWS_PER_CORE = 128 * G  # 25088
S = [1, 3, 6, 10, 15, 21, 28]  # unique monomials per level
OFF = [0, 0, 0, 6, 16, 31, 52]  # device column offset of level l (l>=2)
U = 80  # stored monomials (levels 2..6)
SIZES = (70, 70, 28, 28)  # rows per chunk; consecutive pairs interleave
POPS = 30  # vector ops per pair: 5 levels * (z_a z_b B_a B_b A_a A_b)


def _index_map():
    """Map each of the 1093 reference columns to unique-monomial index 0..83
    (0..3 = [1, x, y, z] host-side; 4+i = device column i)."""
    mono = [[(0, 0, 0)]]
    for l in range(1, L_MAX + 1):
        prev = mono[-1]
        cur = [(a + 1, b, c) for (a, b, c) in prev]
        cur += [(a, b + 1, c) for (a, b, c) in prev[-l:]]
        a, b, c = prev[-1]
        cur += [(a, b, c + 1)]
        mono.append(cur)
    lookup = {t: i for i, t in enumerate(t for lst in mono for t in lst)}
    idx = []
    for l in range(L_MAX + 1):
        for j in range(3**l):
            a = b = c = 0
            for _ in range(l):
                d = j % 3
                j //= 3
                a += d == 0
                b += d == 1
                c += d == 2
            idx.append(lookup[(a, b, c)])
    return np.asarray(idx, dtype=np.intp)


IDX = _index_map()  # [1093] into [1, x, y, z, device cols 0..79]


def _build_nc(sizes=SIZES):
    import concourse.bass as bass
    import concourse.mybir as mybir

    bf16 = mybir.dt.bfloat16
    g = sum(sizes)
    assert g == G
    rows = 128 * g
    starts = np.concatenate([[0], np.cumsum(sizes)[:-1]])
    n_ch = len(sizes)
    assert n_ch % 2 == 0

    nc = bass.Bass()
    dr4 = nc.declare_dram_parameter("dr4", [rows, 4], bf16, isOutput=False)
    # per chunk k the dump is [p, monomial c, row t]: element (p, k, c, t)
    # lands at out[p, starts[k]*U + c*sizes[k] + t]; host untangles
    out = nc.declare_dram_parameter("out", [128, g * U], bf16, isOutput=True)

    dr4_v = dr4[:, :].rearrange("(p g) c -> p (g c)", p=128)  # [128, g*4]

    from contextlib import ExitStack

    with ExitStack() as stack:
        drt = stack.enter_context(nc.sbuf_tensor("drt", [128, g * 4], bf16))
        uq = stack.enter_context(nc.sbuf_tensor("uq", [128, g * U], bf16))
        sem_in = stack.enter_context(nc.semaphore("sem_in"))
        sem_in2 = stack.enter_context(nc.semaphore("sem_in2"))
        sem_out = stack.enter_context(nc.semaphore("sem_out"))
        sem_out2 = stack.enter_context(nc.semaphore("sem_out2"))
        sem_v = stack.enter_context(nc.semaphore("sem_v"))
        block = stack.enter_context(nc.Block(no_gpsimd_drain=True))

        def cview(k):
            # chunk k scratch as [p, monomial, row]
            st, sz = starts[k], sizes[k]
            return uq[:, st * U : (st + sz) * U].rearrange(
                "p (c t) -> p c t", c=U
            )

        def dcomp(k, c0, c1):
            # input components [c0, c1) for chunk k as [p, comp, row]
            st, sz = starts[k], sizes[k]
            return drt[:, st * 4 : (st + sz) * 4].rearrange(
                "p (t c) -> p c t", c=4
            )[:, c0:c1, :]

        # pair op order per level: z_a z_b B_a B_b A_a A_b (6 per level);
        # chunk a completes at its A6 (pair index 29), chunk b at 30
        def cthr(k):
            return POPS * (k // 2) + 29 + (k % 2)

        def store(q, k, sem):
            st, sz = starts[k], sizes[k]
            q.wait_ge(sem_v, cthr(k))
            q.dma_start(
                out=out[:, st * U : (st + sz) * U],
                in_=uq[:, st * U : (st + sz) * U],
            ).then_inc(sem, 16)

        @block.sync
        def _(sync):
            c0 = (sizes[0] + sizes[1]) * 4  # first-pair input columns
            sync.dma_start(out=drt[:, :c0], in_=dr4_v[:, :c0]).then_inc(
                sem_in, 16
            )
            sync.dma_start(out=drt[:, c0:], in_=dr4_v[:, c0:]).then_inc(
                sem_in2, 16
            )
            for k in range(0, n_ch, 2):  # even chunks on the sync queue
                store(sync, k, sem_out)
            sync.wait_ge(sem_out, 16 * (n_ch // 2))

        @block.scalar
        def _(scalar):
            for k in range(1, n_ch, 2):  # odd chunks on the scalar queue
                store(scalar, k, sem_out2)
            scalar.wait_ge(sem_out2, 16 * (n_ch - n_ch // 2))

        @block.vector
        def _(vector):
            vector.wait_ge(sem_in, 16)
            for pair in range(n_ch // 2):
                if pair == 1:
                    vector.wait_ge(sem_in2, 16)
                ks = (2 * pair, 2 * pair + 1)
                vs = [cview(k) for k in ks]
                base = POPS * pair

                for l in range(2, L_MAX + 1):
                    lb = base + 6 * (l - 3)  # previous level's block base
                    o, ps = OFF[l], S[l - 1]
                    zo = o + ps + l  # z^l slot (last of level l)
                    po = OFF[l - 1]
                    pzo = po + ps - 1
                    pto = po + ps - l
                    # z^l = z * z^(l-1)
                    for j, k in enumerate(ks):
                        vv, sz = vs[j], sizes[k]
                        if l == 2:
                            zin = dcomp(k, 3, 4)
                        else:
                            vector.wait_ge(sem_v, lb + 1 + j)
                            zin = vv[:, pzo : pzo + 1, :]
                        vector.tensor_mul(
                            out=vv[:, zo : zo + 1, :],
                            in0=zin,
                            in1=dcomp(k, 3, 4),
                        ).then_inc(sem_v, 1)
                    # y * (a=0 tail of L_{l-1}: its last l entries)
                    for j, k in enumerate(ks):
                        vv, sz = vs[j], sizes[k]
                        if l == 2:
                            ysrc = dcomp(k, 2, 4)
                        else:
                            vector.wait_ge(sem_v, lb + 3 + j)
                            ysrc = vv[:, pto : pto + l, :]
                        vector.tensor_mul(
                            out=vv[:, o + ps : o + ps + l, :],
                            in0=ysrc,
                            in1=dcomp(k, 2, 3).broadcast_to([128, l, sz]),
                        ).then_inc(sem_v, 1)
                    # x * (all of L_{l-1})
                    for j, k in enumerate(ks):
                        vv, sz = vs[j], sizes[k]
                        if l == 2:
                            prev = dcomp(k, 1, 4)
                        else:
                            vector.wait_ge(sem_v, lb + 5 + j)
                            prev = vv[:, po : po + ps, :]
                        vector.tensor_mul(
                            out=vv[:, o : o + ps, :],
                            in0=prev,
                            in1=dcomp(k, 1, 2).broadcast_to([128, ps, sz]),
                        ).then_inc(sem_v, 1)

    return nc


def kernel(dr, _trace=False, _trace_cores=None):
    import ml_dtypes
    from concourse.bass_utils import run_bass_kernel_spmd

    dr = np.asarray(dr, dtype=np.float32)
    n = dr.shape[0]
    # Overlapping shards: core i processes rows [i*step, i*step + 25088) so
    # the 704 rows of pad-to-25088 waste is spread evenly (88 rows per core)
    # instead of all landing on the last core.
    step = n // N_CORES
    assert step <= ROWS_PER_CORE and (N_CORES - 1) * step + ROWS_PER_CORE >= n
    total = (N_CORES - 1) * step + ROWS_PER_CORE
    drb = dr.astype(ml_dtypes.bfloat16)
    dr4 = np.zeros((total, 4), dtype=ml_dtypes.bfloat16)
    dr4[:, 0] = 1.0
    dr4[:n, 1:] = drb

    in_maps = [
        {"dr4": np.ascontiguousarray(dr4[i * step : i * step + ROWS_PER_CORE])}
        for i in range(N_CORES)
    ]
    nc = _build_nc()
    res = run_bass_kernel_spmd(
        nc,
        in_maps,
        core_ids=list(range(N_CORES)),
        trace=_trace,
        trace_cores=_trace_cores,
    )
    kernel.last_result = res

    # untangle the monomial-major per-chunk dumps into [25088, 80] per core
    starts = np.concatenate([[0], np.cumsum(SIZES)[:-1]])
    per_core = []
    for i in range(N_CORES):
        arr = np.asarray(res.results[i]["out"])  # [128, G*U] bf16
        blocks = []
        for k, sz in enumerate(SIZES):
            b = arr[:, starts[k] * U : (starts[k] + sz) * U]
            blocks.append(b.reshape(128, U, sz).transpose(0, 2, 1))
        per_core.append(
            np.concatenate(blocks, axis=1).reshape(ROWS_PER_CORE, U)
        )
    dev = np.concatenate(
        [per_core[i][:step] for i in range(N_CORES - 1)]
        + [per_core[N_CORES - 1][: ROWS_PER_CORE - 88]],
        axis=0,
    )
    # unshard: assemble the 84 unique monomials (host-known [1,x,y,z] +
    # 80 device columns), upcast, and expand to the 1093 output columns
    uniq = np.empty((n, 84), dtype=np.float32)
    uniq[:, 0] = 1.0
    uniq[:, 1:4] = drb.astype(np.float32)  # match device bf16 rounding
    uniq[:, 4:] = dev[:n].astype(np.float32)
    return uniq[:, IDX]
